# revision 3
# baseline (speedup 1.0000x reference)
"""DKVMN (DeepIRT) forward pass on 8 Trainium2 NeuronCores.

Strategy (v2)
-------------
Pure data parallel over the batch (2048 -> 256 per core, 2 partition-tiles
of 128). Token-dependent quantities are folded into gather tables on the
host (weight-only preprocessing):

  Wsoft[q]  = softmax(q_embed @ key_memory^T)   (attention weights w)
  Hq[q]     = q_embed @ pred_w1[V:] + b1        (query part of the MLP)
  Esig[qa]  = sigmoid(qa_embed @ erase_w + be)  (erase gate e)
  Atanh[qa] = tanh(qa_embed @ add_w + ba)       (add vector a)

Sequential scan per step t, per tile (Mv [128, M, V] fp16 SBUF-resident):

  T1  = Mv * w_bc          (DVE TT, w broadcast along v: 2x mode confirmed)
  X2  = T1 * e_bc          (DVE TT, e broadcast along m)
  Mv -= X2                 (DVE TT)
  tree(T1) -> read         (DVE, 7 halving adds, written into chunk buffer)
  Mv += WA                 (DVE TT; WA = w (x) a built by GPSIMD off the
                            critical path, consumed at end of step)

The prediction MLP is batched per K-step chunk on PE/ACT (transposes +
matmuls + tanh/sigmoid), keeping per-step cross-engine sync off the DVE.
"""

import os
import sys

for _p in ("/root/.axon_site/_ro/trn_rl_repo", "/opt/trn_rl_repo"):
    if os.path.isdir(_p) and _p not in sys.path:
        sys.path.append(_p)

import numpy as np

import concourse.bacc as bacc
import concourse.bass as bass
import concourse.tile as tile
from concourse import mybir
from concourse.bass_utils import run_bass_kernel_spmd
from concourse.masks import make_identity

# Problem shapes (hardcoded per harness contract)
B, S, M, V, KD, FC = 2048, 200, 50, 200, 50, 50
NQ, NQA = 5001, 10001
NCORES = 8
BL = B // NCORES      # 256 batch rows per core
P = 128               # SBUF partitions
NT = BL // P          # 2 batch tiles per core
KSTEPS = 2            # time steps per gather block
EAW = 512             # ea-table row width (fp16 elems); 1024B, %256 ok
WHW = 128             # wh-table row width; 256B
IDX_PER_BLK = BL * KSTEPS        # 512 gathered rows per block per table
IDXCOLS = BL * S // 16           # wrapped idx array columns
KC = 8                # MLP chunk length (steps); S % KC == 0

_prog_cache = {}


def _build_program(steps=S):
    dt = mybir.dt
    nc = bacc.Bacc("TRN2", debug=False)

    ea_t = nc.dram_tensor("ea_table", [NQA, EAW], dt.float16, kind="ExternalInput")
    wh_t = nc.dram_tensor("wh_table", [NQ, WHW], dt.float16, kind="ExternalInput")
    w1r_d = nc.dram_tensor("w1r", [2, 100, FC], dt.float16, kind="ExternalInput")
    w2_d = nc.dram_tensor("w2rep", [P, FC], dt.float16, kind="ExternalInput")
    b2_d = nc.dram_tensor("b2rep", [P, 1], dt.float32, kind="ExternalInput")
    mv_d = nc.dram_tensor("mv_init", [1, M * V], dt.float16, kind="ExternalInput")
    qi_d = nc.dram_tensor("qidx", [P, IDXCOLS], dt.int16, kind="ExternalInput")
    qa_d = nc.dram_tensor("qaidx", [P, IDXCOLS], dt.int16, kind="ExternalInput")
    preds_d = nc.dram_tensor("preds_out", [BL, S], dt.float32, kind="ExternalOutput")

    nblk = steps // KSTEPS
    nchunk = (steps + KC - 1) // KC

    from contextlib import ExitStack

    with tile.TileContext(nc) as tc, ExitStack() as ctx:
        consts = ctx.enter_context(tc.tile_pool(name="consts", bufs=1))
        state = ctx.enter_context(tc.tile_pool(name="state", bufs=1))
        gath = ctx.enter_context(tc.tile_pool(name="gath", bufs=2))
        small = ctx.enter_context(tc.tile_pool(name="small", bufs=2))
        psum = ctx.enter_context(tc.tile_pool(name="psum", bufs=2, space="PSUM"))
        psmm = ctx.enter_context(tc.tile_pool(name="psmm", bufs=2, space="PSUM"))

        mult = mybir.AluOpType.mult
        addop = mybir.AluOpType.add

        # ---- constants ----
        w1r_sb = consts.tile([100, 2, FC], dt.float16)
        for c in range(2):
            nc.sync.dma_start(out=w1r_sb[:, c, :], in_=w1r_d[c])
        w2_sb = consts.tile([P, FC], dt.float16)
        nc.sync.dma_start(out=w2_sb[:], in_=w2_d[:])
        b2_sb = consts.tile([P, 1], dt.float32)
        nc.sync.dma_start(out=b2_sb[:], in_=b2_d[:])
        ident = consts.tile([P, P], dt.float16)
        make_identity(nc, ident)

        # ---- persistent state ----
        Mvs, T1s, X2s, WAs, RBs, HQs, RTs = [], [], [], [], [], [], []
        for tl in range(NT):
            Mv = state.tile([P, M, V], dt.float16, tag=f"mv{tl}", name=f"mv{tl}")
            nc.sync.dma_start(
                out=Mv[:].rearrange("p m v -> p (m v)"),
                in_=mv_d[:].to_broadcast((P, M * V)),
            )
            Mvs.append(Mv)
            T1s.append(state.tile([P, M, V], dt.float16, tag=f"t1{tl}", name=f"t1{tl}"))
            X2s.append(state.tile([P, M, V], dt.float16, tag=f"x2{tl}", name=f"x2{tl}"))
            WAs.append(state.tile([P, M, V], dt.float16, tag=f"wa{tl}", name=f"wa{tl}"))
            # read buffer: double-buffered chunks [parity, KC, V]
            RBs.append(state.tile([P, 2, KC, V], dt.float16, tag=f"rb{tl}", name=f"rb{tl}"))
            HQs.append(state.tile([P, 2, KC, FC], dt.float16, tag=f"hq{tl}", name=f"hq{tl}"))
            RTs.append(state.tile([100, KC, 2, P], dt.float16, tag=f"rt{tl}", name=f"rt{tl}"))
        preds_buf = state.tile([P, NT, S], dt.float32, tag="preds")

        def flush_chunk(c0, klen, par):
            """Batched MLP for steps [c0, c0+klen) using readbuf parity `par`."""
            for tl in range(NT):
                RB, HQ, RT = RBs[tl], HQs[tl], RTs[tl]
                h_ps = psmm.tile([P, KC, FC], dt.float32, tag=f"hps{tl}")
                for kk in range(klen):
                    for h in range(2):
                        pT = psum.tile([100, P], dt.float16, tag="pT")
                        nc.tensor.transpose(
                            pT[:], RB[:, par, kk, h * 100:(h + 1) * 100], ident[:])
                        nc.scalar.copy(RT[:, kk, h, :], pT[:])
                    nc.tensor.matmul(h_ps[:, kk, :], lhsT=RT[:, kk, 0, :],
                                     rhs=w1r_sb[:, 0, :], start=True, stop=False)
                    nc.tensor.matmul(h_ps[:, kk, :], lhsT=RT[:, kk, 1, :],
                                     rhs=w1r_sb[:, 1, :], start=False, stop=True)
                hh = small.tile([P, KC, FC], dt.float16, tag="hh")
                nc.vector.tensor_add(hh[:, 0:klen, :], h_ps[:, 0:klen, :],
                                     HQ[:, par, 0:klen, :])
                nc.scalar.activation(hh[:, 0:klen, :], hh[:, 0:klen, :],
                                     mybir.ActivationFunctionType.Tanh)
                hw2 = small.tile([P, KC, FC], dt.float16, tag="hw2")
                nc.vector.tensor_mul(
                    hw2[:, 0:klen, :], hh[:, 0:klen, :],
                    w2_sb[:, None, :].to_broadcast((P, klen, FC)))
                pacc = small.tile([P, KC], dt.float32, tag="pacc")
                nc.vector.tensor_reduce(pacc[:, 0:klen], hw2[:, 0:klen, :],
                                        mybir.AxisListType.X, addop)
                nc.scalar.activation(
                    preds_buf[:, tl, c0:c0 + klen], pacc[:, 0:klen],
                    mybir.ActivationFunctionType.Sigmoid, bias=b2_sb[:],
                )

        # ---- scan ----
        for g in range(nblk):
            qi = gath.tile([P, IDX_PER_BLK // 16], dt.int16, tag="qi")
            qa = gath.tile([P, IDX_PER_BLK // 16], dt.int16, tag="qa")
            c0 = g * (IDX_PER_BLK // 16)
            nc.sync.dma_start(out=qi[:], in_=qi_d[:, c0:c0 + IDX_PER_BLK // 16])
            nc.sync.dma_start(out=qa[:], in_=qa_d[:, c0:c0 + IDX_PER_BLK // 16])
            ea_blk = gath.tile([P, NT * KSTEPS, EAW], dt.float16, tag="ea")
            wh_blk = gath.tile([P, NT * KSTEPS, WHW], dt.float16, tag="wh")
            nc.gpsimd.dma_gather(ea_blk[:], ea_t[:], qa[:], IDX_PER_BLK, IDX_PER_BLK, EAW)
            nc.gpsimd.dma_gather(wh_blk[:], wh_t[:], qi[:], IDX_PER_BLK, IDX_PER_BLK, WHW)

            # stash hq rows for the chunked MLP
            for k in range(KSTEPS):
                t = g * KSTEPS + k
                kk, par = t % KC, (t // KC) % 2
                for tl in range(NT):
                    c = k * NT + tl
                    nc.scalar.copy(HQs[tl][:, par, kk, :], wh_blk[:, c, 64:64 + FC])

            for k in range(KSTEPS):
                t = g * KSTEPS + k
                kk, par = t % KC, (t // KC) % 2
                for tl in range(NT):
                    c = k * NT + tl
                    w_bc = wh_blk[:, c, 0:M][:, :, None].to_broadcast((P, M, V))
                    e_bc = ea_blk[:, c, 0:V][:, None, :].to_broadcast((P, M, V))
                    a_bc = ea_blk[:, c, 256:256 + V][:, None, :].to_broadcast((P, M, V))
                    Mv, T1, X2, WA = Mvs[tl], T1s[tl], X2s[tl], WAs[tl]

                    # GPSIMD: WA = w (x) a  (both operands broadcast views)
                    nc.gpsimd.tensor_mul(WA[:], w_bc, a_bc)
                    # DVE: T1 = Mv * w_rep  (weighted memory, pre-update)
                    nc.vector.tensor_mul(T1[:], Mv[:], w_bc)
                    # DVE: X2 = T1 * e_bc (erase term), Mv -= X2
                    nc.vector.tensor_mul(X2[:], T1[:], e_bc)
                    nc.vector.tensor_sub(Mv[:], Mv[:], X2[:])
                    # DVE: add-tree over m on T1 (in place) -> read row
                    nc.vector.tensor_add(T1[:, 0:25, :], T1[:, 0:25, :], T1[:, 25:50, :])
                    nc.vector.tensor_add(T1[:, 0:12, :], T1[:, 0:12, :], T1[:, 12:24, :])
                    nc.vector.tensor_add(T1[:, 0:6, :], T1[:, 0:6, :], T1[:, 6:12, :])
                    nc.vector.tensor_add(T1[:, 0:3, :], T1[:, 0:3, :], T1[:, 3:6, :])
                    nc.vector.tensor_add(T1[:, 0:1, :], T1[:, 0:1, :], T1[:, 1:2, :])
                    nc.vector.tensor_add(T1[:, 0:1, :], T1[:, 0:1, :], T1[:, 2:3, :])
                    nc.vector.tensor_add(RBs[tl][:, par, kk, :], T1[:, 0, :], T1[:, 24, :])
                # end-of-step: apply the add-vector updates (GPS had the
                # whole step to produce WA)
                for tl in range(NT):
                    nc.vector.tensor_add(Mvs[tl][:], Mvs[tl][:], WAs[tl][:])
                if kk == KC - 1:
                    flush_chunk(t - KC + 1, KC, par)
            # tail chunk
        if steps % KC:
            t = steps - 1
            flush_chunk(steps - steps % KC, steps % KC, (t // KC) % 2)

        # ---- write out ----
        pv = preds_d[:].rearrange("(n p) s -> n p s", p=P)
        for tl in range(NT):
            nc.sync.dma_start(out=pv[tl][:, 0:steps], in_=preds_buf[:, tl, 0:steps])

    nc.finalize()
    return nc


def _wrap_idx(seq):
    """seq [N] -> [128, N//16] int16 wrapped (idx i at [i%16, i//16], 8x replicated)."""
    n = seq.shape[0]
    arr16 = seq.reshape(n // 16, 16).T.astype(np.int16)
    return np.tile(arr16, (8, 1))


def _host_tables(inputs):
    f32 = np.float32
    qe = inputs["q_embed_w"].astype(f32)
    qae = inputs["qa_embed_w"].astype(f32)
    km = inputs["key_memory"].astype(f32)

    logits = qe @ km.T
    ex = np.exp(logits - logits.max(-1, keepdims=True))
    wsoft = ex / ex.sum(-1, keepdims=True)
    hq = qe @ inputs["pred_w1"][V:, :].astype(f32) + inputs["pred_b1"].astype(f32)
    esig = 1.0 / (1.0 + np.exp(-(qae @ inputs["erase_w"].astype(f32) + inputs["erase_b"].astype(f32))))
    atanh = np.tanh(qae @ inputs["add_w"].astype(f32) + inputs["add_b"].astype(f32))

    ea = np.zeros((NQA, EAW), np.float16)
    ea[:, 0:V] = esig.astype(np.float16)
    ea[:, 256:256 + V] = atanh.astype(np.float16)
    wh = np.zeros((NQ, WHW), np.float16)
    wh[:, 0:M] = wsoft.astype(np.float16)
    wh[:, 64:64 + FC] = hq.astype(np.float16)

    w1r = inputs["pred_w1"][:V, :].astype(np.float16).reshape(2, 100, FC)
    w2rep = np.tile(inputs["pred_w2"][:, 0].astype(np.float16)[None, :], (P, 1))
    b2rep = np.full((P, 1), inputs["pred_b2"][0], np.float32)
    mv_init = inputs["init_value_memory"].astype(np.float16).reshape(1, -1)
    return dict(ea_table=ea, wh_table=wh, w1r=w1r, w2rep=w2rep, b2rep=b2rep,
                mv_init=mv_init)


def kernel(**inputs):
    inputs = {k: np.asarray(v) for k, v in inputs.items()}
    steps = int(os.environ.get("KERNEL_STEPS", S))

    if steps not in _prog_cache:
        _prog_cache[steps] = _build_program(steps)
    nc = _prog_cache[steps]

    shared = _host_tables(inputs)
    q = inputs["q_data"].astype(np.int64)
    qa = inputs["qa_data"].astype(np.int64)

    in_maps = []
    for core in range(NCORES):
        qs = q[core * BL:(core + 1) * BL]       # [256, S]
        qas = qa[core * BL:(core + 1) * BL]
        # gather order: block g, step k, tile tl, partition p
        #   -> element (g*K + k) of column (tl*128+p)
        def order(x):
            # x [BL, S] -> [S, NT, P] -> [NBLK, KSTEPS, NT, P] flat
            xt = x.T.reshape(S, NT, P)
            return xt.reshape(S // KSTEPS, KSTEPS, NT, P).reshape(-1)
        m = dict(shared)
        m["qidx"] = _wrap_idx(order(qs))
        m["qaidx"] = _wrap_idx(order(qas))
        in_maps.append(m)

    trace = bool(int(os.environ.get("KERNEL_TRACE", "0")))
    res = run_bass_kernel_spmd(nc, in_maps, core_ids=list(range(NCORES)), trace=trace)
    global LAST_RESULTS
    LAST_RESULTS = res
    preds = np.concatenate(
        [res.results[i]["preds_out"] for i in range(NCORES)], axis=0
    ).astype(np.float32)
    z = np.zeros_like(preds)
    return (preds, z, z, z)


# revision 7
# speedup vs baseline: 1.0524x; 1.0524x over previous
"""DKVMN (DeepIRT) forward pass on 8 Trainium2 NeuronCores.

Strategy (v2)
-------------
Pure data parallel over the batch (2048 -> 256 per core, 2 partition-tiles
of 128). Token-dependent quantities are folded into gather tables on the
host (weight-only preprocessing):

  Wsoft[q]  = softmax(q_embed @ key_memory^T)   (attention weights w)
  Hq[q]     = q_embed @ pred_w1[V:] + b1        (query part of the MLP)
  Esig[qa]  = sigmoid(qa_embed @ erase_w + be)  (erase gate e)
  Atanh[qa] = tanh(qa_embed @ add_w + ba)       (add vector a)

Sequential scan per step t, per tile (Mv [128, M, V] fp16 SBUF-resident):

  T1  = Mv * w_bc          (DVE TT, w broadcast along v: 2x mode confirmed)
  X2  = T1 * e_bc          (DVE TT, e broadcast along m)
  Mv -= X2                 (DVE TT)
  tree(T1) -> read         (DVE, 7 halving adds, written into chunk buffer)
  Mv += WA                 (DVE TT; WA = w (x) a built by GPSIMD off the
                            critical path, consumed at end of step)

The prediction MLP is batched per K-step chunk on PE/ACT (transposes +
matmuls + tanh/sigmoid), keeping per-step cross-engine sync off the DVE.
"""

import os
import sys

for _p in ("/root/.axon_site/_ro/trn_rl_repo", "/opt/trn_rl_repo"):
    if os.path.isdir(_p) and _p not in sys.path:
        sys.path.append(_p)

import numpy as np

import concourse.bacc as bacc
import concourse.bass as bass
import concourse.tile as tile
from concourse import mybir
from concourse.bass_utils import run_bass_kernel_spmd
from concourse.masks import make_identity

# Problem shapes (hardcoded per harness contract)
B, S, M, V, KD, FC = 2048, 200, 50, 200, 50, 50
NQ, NQA = 5001, 10001
NCORES = 8
BL = B // NCORES      # 256 batch rows per core
P = 128               # SBUF partitions
NT = BL // P          # 2 batch tiles per core
KSTEPS = 2            # time steps per gather block
EAW = 512             # ea-table row width (fp16 elems); 1024B, %256 ok
WHW = 128             # wh-table row width; 256B
IDX_PER_BLK = BL * KSTEPS        # 512 gathered rows per block per table
IDXCOLS = BL * S // 16           # wrapped idx array columns
KC = 8                # MLP chunk length (steps); S % KC == 0

_prog_cache = {}


def _build_program(steps=S):
    dt = mybir.dt
    nc = bacc.Bacc("TRN2", debug=False)

    ea_t = nc.dram_tensor("ea_table", [NQA, EAW], dt.float16, kind="ExternalInput")
    wh_t = nc.dram_tensor("wh_table", [NQ, WHW], dt.float16, kind="ExternalInput")
    w1r_d = nc.dram_tensor("w1r", [2, 100, FC], dt.float16, kind="ExternalInput")
    w2_d = nc.dram_tensor("w2rep", [P, FC], dt.float16, kind="ExternalInput")
    b2_d = nc.dram_tensor("b2rep", [P, 1], dt.float32, kind="ExternalInput")
    mv_d = nc.dram_tensor("mv_init", [1, M * V], dt.float16, kind="ExternalInput")
    qi_d = nc.dram_tensor("qidx", [P, IDXCOLS], dt.int16, kind="ExternalInput")
    qa_d = nc.dram_tensor("qaidx", [P, IDXCOLS], dt.int16, kind="ExternalInput")
    preds_d = nc.dram_tensor("preds_out", [BL, S], dt.float32, kind="ExternalOutput")

    nblk = steps // KSTEPS
    nchunk = (steps + KC - 1) // KC

    from contextlib import ExitStack

    with tile.TileContext(nc) as tc, ExitStack() as ctx:
        consts = ctx.enter_context(tc.tile_pool(name="consts", bufs=1))
        state = ctx.enter_context(tc.tile_pool(name="state", bufs=1))
        gath = ctx.enter_context(tc.tile_pool(name="gath", bufs=2))
        small = ctx.enter_context(tc.tile_pool(name="small", bufs=2))
        psum = ctx.enter_context(tc.tile_pool(name="psum", bufs=2, space="PSUM"))
        psmm = ctx.enter_context(tc.tile_pool(name="psmm", bufs=2, space="PSUM"))

        mult = mybir.AluOpType.mult
        addop = mybir.AluOpType.add

        # ---- constants ----
        w1r_sb = consts.tile([100, 2, FC], dt.float16)
        for c in range(2):
            nc.sync.dma_start(out=w1r_sb[:, c, :], in_=w1r_d[c])
        w2_sb = consts.tile([P, FC], dt.float16)
        nc.sync.dma_start(out=w2_sb[:], in_=w2_d[:])
        b2_sb = consts.tile([P, 1], dt.float32)
        nc.sync.dma_start(out=b2_sb[:], in_=b2_d[:])
        ident = consts.tile([P, P], dt.float16)
        make_identity(nc, ident)

        # ---- persistent state ----
        Mvs, T1s, X2s, WAs, RBs, HQs, RTs = [], [], [], [], [], [], []
        for tl in range(NT):
            Mv = state.tile([P, M, V], dt.float16, tag=f"mv{tl}", name=f"mv{tl}")
            nc.sync.dma_start(
                out=Mv[:].rearrange("p m v -> p (m v)"),
                in_=mv_d[:].to_broadcast((P, M * V)),
            )
            Mvs.append(Mv)
            T1s.append(state.tile([P, M, V], dt.float16, tag=f"t1{tl}", name=f"t1{tl}"))
            X2s.append(state.tile([P, M, V], dt.float16, tag=f"x2{tl}", name=f"x2{tl}"))
            WAs.append(state.tile([P, M, V], dt.float16, tag=f"wa{tl}", name=f"wa{tl}"))
            # read buffer: double-buffered chunks [parity, KC, V]
            RBs.append(state.tile([P, 2, KC, V], dt.float16, tag=f"rb{tl}", name=f"rb{tl}"))
            HQs.append(state.tile([P, 2, KC, FC], dt.float16, tag=f"hq{tl}", name=f"hq{tl}"))
            RTs.append(state.tile([100, KC, 2, P], dt.float16, tag=f"rt{tl}", name=f"rt{tl}"))
        preds_buf = state.tile([P, NT, S], dt.float32, tag="preds")

        def flush_chunk(c0, klen, par):
            """Batched MLP for steps [c0, c0+klen) using readbuf parity `par`."""
            for tl in range(NT):
                RB, HQ, RT = RBs[tl], HQs[tl], RTs[tl]
                h_ps = psmm.tile([P, KC, FC], dt.float32, tag=f"hps{tl}")
                for kk in range(klen):
                    for h in range(2):
                        pT = psum.tile([100, P], dt.float16, tag="pT")
                        nc.tensor.transpose(
                            pT[:], RB[:, par, kk, h * 100:(h + 1) * 100], ident[:])
                        nc.scalar.copy(RT[:, kk, h, :], pT[:])
                    nc.tensor.matmul(h_ps[:, kk, :], lhsT=RT[:, kk, 0, :],
                                     rhs=w1r_sb[:, 0, :], start=True, stop=False)
                    nc.tensor.matmul(h_ps[:, kk, :], lhsT=RT[:, kk, 1, :],
                                     rhs=w1r_sb[:, 1, :], start=False, stop=True)
                hh = small.tile([P, KC, FC], dt.float16, tag="hh")
                nc.vector.tensor_add(hh[:, 0:klen, :], h_ps[:, 0:klen, :],
                                     HQ[:, par, 0:klen, :])
                nc.scalar.activation(hh[:, 0:klen, :], hh[:, 0:klen, :],
                                     mybir.ActivationFunctionType.Tanh)
                hw2 = small.tile([P, KC, FC], dt.float16, tag="hw2")
                nc.vector.tensor_mul(
                    hw2[:, 0:klen, :], hh[:, 0:klen, :],
                    w2_sb[:, None, :].to_broadcast((P, klen, FC)))
                pacc = small.tile([P, KC], dt.float32, tag="pacc")
                nc.vector.tensor_reduce(pacc[:, 0:klen], hw2[:, 0:klen, :],
                                        mybir.AxisListType.X, addop)
                nc.scalar.activation(
                    preds_buf[:, tl, c0:c0 + klen], pacc[:, 0:klen],
                    mybir.ActivationFunctionType.Sigmoid, bias=b2_sb[:],
                )

        # ---- scan ----
        for g in range(nblk):
            qi = gath.tile([P, IDX_PER_BLK // 16], dt.int16, tag="qi")
            qa = gath.tile([P, IDX_PER_BLK // 16], dt.int16, tag="qa")
            c0 = g * (IDX_PER_BLK // 16)
            nc.sync.dma_start(out=qi[:], in_=qi_d[:, c0:c0 + IDX_PER_BLK // 16])
            nc.sync.dma_start(out=qa[:], in_=qa_d[:, c0:c0 + IDX_PER_BLK // 16])
            ea_blk = gath.tile([P, NT * KSTEPS, EAW], dt.float16, tag="ea")
            wh_blk = gath.tile([P, NT * KSTEPS, WHW], dt.float16, tag="wh")
            nc.gpsimd.dma_gather(ea_blk[:], ea_t[:], qa[:], IDX_PER_BLK, IDX_PER_BLK, EAW)
            nc.gpsimd.dma_gather(wh_blk[:], wh_t[:], qi[:], IDX_PER_BLK, IDX_PER_BLK, WHW)

            # stash hq rows for the chunked MLP; fp32 copy of w for ACT scale
            wf32 = gath.tile([P, NT * KSTEPS, M], dt.float32, tag="wf32")
            nc.scalar.copy(wf32[:], wh_blk[:, :, 0:M])
            for k in range(KSTEPS):
                t = g * KSTEPS + k
                kk, par = t % KC, (t // KC) % 2
                for tl in range(NT):
                    c = k * NT + tl
                    nc.scalar.copy(HQs[tl][:, par, kk, :], wh_blk[:, c, 64:64 + FC])

            for k in range(KSTEPS):
                t = g * KSTEPS + k
                kk, par = t % KC, (t // KC) % 2
                cs = [k * NT + tl for tl in range(NT)]
                w_bcs = [wh_blk[:, c, 0:M][:, :, None].to_broadcast((P, M, V)) for c in cs]
                e_bcs = [ea_blk[:, c, 0:V][:, None, :].to_broadcast((P, M, V)) for c in cs]
                a_rows = [ea_blk[:, c, 256:256 + V] for c in cs]

                # ACT: WA_tl = w (x) a via 50 per-slot scaled copies (idle engine)
                for tl in range(NT):
                    for m in range(M):
                        nc.scalar.activation(
                            WAs[tl][:, m, :], a_rows[tl],
                            mybir.ActivationFunctionType.Copy,
                            scale=wf32[:, cs[tl], m:m + 1])

                def tree(tl):
                    T1 = T1s[tl]
                    nc.vector.tensor_add(T1[:, 0:25, :], T1[:, 0:25, :], T1[:, 25:50, :])
                    nc.vector.tensor_add(T1[:, 0:12, :], T1[:, 0:12, :], T1[:, 12:24, :])
                    nc.vector.tensor_add(T1[:, 0:6, :], T1[:, 0:6, :], T1[:, 6:12, :])
                    nc.vector.tensor_add(T1[:, 0:3, :], T1[:, 0:3, :], T1[:, 3:6, :])
                    nc.vector.tensor_add(T1[:, 0:1, :], T1[:, 0:1, :], T1[:, 1:2, :])
                    nc.vector.tensor_add(T1[:, 0:1, :], T1[:, 0:1, :], T1[:, 2:3, :])
                    nc.vector.tensor_add(RBs[tl][:, par, kk, :], T1[:, 0, :], T1[:, 24, :])

                # DVE: T1 = Mv * w_rep (pre-update); tile0 first so GPSIMD
                # can start on X2_0 early
                nc.vector.tensor_mul(T1s[0][:], Mvs[0][:], w_bcs[0])
                # GPSIMD: erase term for tile 0
                nc.gpsimd.tensor_mul(X2s[0][:], T1s[0][:], e_bcs[0])
                nc.vector.tensor_mul(T1s[1][:], Mvs[1][:], w_bcs[1])
                # DVE: erase term + update for tile 1
                nc.vector.tensor_mul(X2s[1][:], T1s[1][:], e_bcs[1])
                nc.vector.tensor_sub(Mvs[1][:], Mvs[1][:], X2s[1][:])
                # GPSIMD: tile-1 add-vector update (runs while DVE trees)
                nc.gpsimd.tensor_add(Mvs[1][:], Mvs[1][:], WAs[1][:])
                tree(1)
                # tile 0 update (GPSIMD X2_0 ready by now)
                nc.vector.tensor_sub(Mvs[0][:], Mvs[0][:], X2s[0][:])
                tree(0)
                # end-of-step tile-0 add-vector update (ACT had the whole step)
                nc.vector.tensor_add(Mvs[0][:], Mvs[0][:], WAs[0][:])
                if kk == KC - 1:
                    flush_chunk(t - KC + 1, KC, par)
            # tail chunk
        if steps % KC:
            t = steps - 1
            flush_chunk(steps - steps % KC, steps % KC, (t // KC) % 2)

        # ---- write out ----
        pv = preds_d[:].rearrange("(n p) s -> n p s", p=P)
        for tl in range(NT):
            nc.sync.dma_start(out=pv[tl][:, 0:steps], in_=preds_buf[:, tl, 0:steps])

    nc.finalize()
    return nc


def _wrap_idx(seq):
    """seq [N] -> [128, N//16] int16 wrapped (idx i at [i%16, i//16], 8x replicated)."""
    n = seq.shape[0]
    arr16 = seq.reshape(n // 16, 16).T.astype(np.int16)
    return np.tile(arr16, (8, 1))


def _host_tables(inputs):
    f32 = np.float32
    qe = inputs["q_embed_w"].astype(f32)
    qae = inputs["qa_embed_w"].astype(f32)
    km = inputs["key_memory"].astype(f32)

    logits = qe @ km.T
    ex = np.exp(logits - logits.max(-1, keepdims=True))
    wsoft = ex / ex.sum(-1, keepdims=True)
    hq = qe @ inputs["pred_w1"][V:, :].astype(f32) + inputs["pred_b1"].astype(f32)
    esig = 1.0 / (1.0 + np.exp(-(qae @ inputs["erase_w"].astype(f32) + inputs["erase_b"].astype(f32))))
    atanh = np.tanh(qae @ inputs["add_w"].astype(f32) + inputs["add_b"].astype(f32))

    ea = np.zeros((NQA, EAW), np.float16)
    ea[:, 0:V] = esig.astype(np.float16)
    ea[:, 256:256 + V] = atanh.astype(np.float16)
    wh = np.zeros((NQ, WHW), np.float16)
    wh[:, 0:M] = wsoft.astype(np.float16)
    wh[:, 64:64 + FC] = hq.astype(np.float16)

    w1r = inputs["pred_w1"][:V, :].astype(np.float16).reshape(2, 100, FC)
    w2rep = np.tile(inputs["pred_w2"][:, 0].astype(np.float16)[None, :], (P, 1))
    b2rep = np.full((P, 1), inputs["pred_b2"][0], np.float32)
    mv_init = inputs["init_value_memory"].astype(np.float16).reshape(1, -1)
    return dict(ea_table=ea, wh_table=wh, w1r=w1r, w2rep=w2rep, b2rep=b2rep,
                mv_init=mv_init)


def kernel(**inputs):
    inputs = {k: np.asarray(v) for k, v in inputs.items()}
    steps = int(os.environ.get("KERNEL_STEPS", S))

    if steps not in _prog_cache:
        _prog_cache[steps] = _build_program(steps)
    nc = _prog_cache[steps]

    shared = _host_tables(inputs)
    q = inputs["q_data"].astype(np.int64)
    qa = inputs["qa_data"].astype(np.int64)

    in_maps = []
    for core in range(NCORES):
        qs = q[core * BL:(core + 1) * BL]       # [256, S]
        qas = qa[core * BL:(core + 1) * BL]
        # gather order: block g, step k, tile tl, partition p
        #   -> element (g*K + k) of column (tl*128+p)
        def order(x):
            # x [BL, S] -> [S, NT, P] -> [NBLK, KSTEPS, NT, P] flat
            xt = x.T.reshape(S, NT, P)
            return xt.reshape(S // KSTEPS, KSTEPS, NT, P).reshape(-1)
        m = dict(shared)
        m["qidx"] = _wrap_idx(order(qs))
        m["qaidx"] = _wrap_idx(order(qas))
        in_maps.append(m)

    trace = bool(int(os.environ.get("KERNEL_TRACE", "0")))
    res = run_bass_kernel_spmd(nc, in_maps, core_ids=list(range(NCORES)), trace=trace)
    global LAST_RESULTS
    LAST_RESULTS = res
    preds = np.concatenate(
        [res.results[i]["preds_out"] for i in range(NCORES)], axis=0
    ).astype(np.float32)
    z = np.zeros_like(preds)
    return (preds, z, z, z)
